# revision 1
# baseline (speedup 1.0000x reference)
"""BERT self-attention kernel for Trainium2, sharded over 8 NeuronCores.

Problem: nn_CustomBertSelfAttention (B=2, S=2048, D=1024, H=16 heads, HD=64).

Sharding: tensor-parallel over heads. Core c owns heads {2c, 2c+1}, i.e.
columns [128c, 128c+128) of Wq/Wk/Wv and of the output. Every core reads the
full hidden_states (transposed + cast to bf16 on the host so the contraction
dim lands on SBUF partitions with dense DMA).

Per-core pipeline (all matmuls bf16 with f32 PSUM accumulation):
  1. Projections Q^T/K^T/V^T [128, BS] = W^T @ x^T as six uniform sections
     (q/k/v x batch) on a ring of psum tiles — no pool barriers. Q/K get
     their bias on DVE during eviction; the V bias is applied on the host.
     V^T is PE-transposed back to V [keys, hd] and stored interleaved:
     vv[b] = [V_h0(64) | 1 | V_h1(64) | 1] per key tile, so each unit's
     augmented stationary [V|1] is one contiguous 65-column slice.
  2. Attention per unit (b, h), key-tile-outer with both 1024-wide query
     halves as lanes: scores^T [keys, q] = K_tile^T.T @ Q^T for lane 0 and 1
     (4 matmuls sharing one stationary load), exp on ScalarE with the
     additive attention mask folded in as the activation's per-partition
     bias (exact: exp(s*sc + m) = e^m e^{s*sc}), then
     ctx^T [65, q] += [V|1]^T @ P^T for both lanes (again one stationary
     load), accumulated over key tiles. Row 64 is the softmax denominator.
     No on-device normalization: the raw [65, S] goes to DRAM and the host
     divides (and adds the V bias).
  3. A post-build IR pass drops InstLdweights whose stationary is identical
     to the one already loaded, removing redundant ~100ns PE weight reloads
     the tile framework emits per matmul. The tile scheduler overlaps the
     projection tail with early attention on its own.
Host: out[u] = (ctx[0:64] / ctx[64])^T + bv  gathered into [B, S, D].
"""
import sys

sys.path.insert(0, "/opt/trn_rl_repo")

import numpy as np
import ml_dtypes

from concourse import bacc
import concourse.mybir as mybir
from concourse.tile import TileContext
from concourse.masks import make_identity
from concourse.bass_utils import run_bass_kernel_spmd

B, S, D, H, HD = 2, 2048, 1024, 16, 64
N_CORES = 8
HPC = H // N_CORES          # heads per core = 2
DC = D // N_CORES           # output/weight columns per core = 128
BS = B * S                  # 4096
NU = B * HPC                # attention units per core = 4
P = 128
F32 = mybir.dt.float32
BF16 = mybir.dt.bfloat16
KT = S // P                 # 16 key tiles per unit
QH = 1024                   # query lane width
NL = S // QH                # 2 query lanes per unit
SCH = 1024                  # projection chunk (BS columns per psum tile)
W65 = HD + 1                # V_aug width (V columns + ones column)
W130 = 2 * W65              # two heads interleaved per key tile in vv[b]
DT = D // P                 # 8 contraction tiles
SCALE = float(1.0 / np.sqrt(HD))

DEDUPE_LDWEIGHTS = True

_cached_nc = None


def _ap_key(arg):
    """Stable identity key for an LDWEIGHTS stationary access pattern."""
    try:
        bass_ap = getattr(arg, "bass_ap", None)
        if bass_ap is not None:
            return ("bap", bass_ap.tensor.name, bass_ap.offset,
                    tuple(map(tuple, bass_ap.ap)), str(arg.dtype))
        return ("raw", getattr(arg, "memref", ""), arg.offset,
                tuple(map(tuple, arg.ap)), str(arg.dtype))
    except Exception:
        return ("repr", repr(arg))


def _dedupe_ldweights(nc, keep_mm_names=()):
    """Drop PE weight reloads whose stationary is already in the array.

    The tile legalizer splits every InstMatmult into InstLdweights +
    InstMatmult. Runs of matmuls that share a stationary reload it
    redundantly; the PE array retains the stationary across matmuls, so
    duplicate loads are pure overhead (~100ns each). Dependencies carried
    by a dropped load are merged into the next PE instruction so no
    synchronization is lost. Operates on the post-scheduler order, so only
    loads that are genuinely redundant at execution time are removed.

    Loads whose matmul name is in ``keep_mm_names`` are preserved even when
    redundant — deliberate PE "keep-warm" work that plugs periodic idle
    gaps which would otherwise drop the PE p-state.
    """
    keep = set(keep_mm_names)
    pe = mybir.EngineType.PE
    for f in nc.m.functions:
        for blk in f.blocks:
            insts = blk.instructions
            pe_seq = [i for i in insts
                      if getattr(i, "engine", None) == pe]
            # matmul each LD self-loads for = next non-LD PE instruction
            next_mm_name = {}
            pending = []
            for i in pe_seq:
                if type(i).__name__ == "InstLdweights":
                    pending.append(i)
                else:
                    for ld in pending:
                        next_mm_name[id(ld)] = i.name
                    pending = []
            drop = set()
            cur_key = None
            pending_merge = []  # deps from dropped LDs awaiting next PE inst
            for i in pe_seq:
                tn = type(i).__name__
                if tn == "InstLdweights":
                    key = (
                        _ap_key(i.ins[0]),
                        getattr(i, "is_transpose", None),
                        getattr(i, "perf_mode", None),
                        getattr(i, "tile_position", None),
                    )
                    if key == cur_key and next_mm_name.get(id(i)) not in keep:
                        drop.add(id(i))
                        pending_merge.append(i)
                    else:
                        cur_key = key
                elif pending_merge:
                    for ld in pending_merge:
                        i.merge_dependencies_from(ld)
                    pending_merge = []
            if drop:
                blk.instructions = [i for i in insts if id(i) not in drop]


def _mm_pair(nc, ps, lhsT, rhs0, rhs1, start, stop):
    """Two n=512 matmuls sharing one stationary (reload deduped later).
    Returns the IR names of the two matmuls."""
    m1 = nc.tensor.matmul(ps[:, 0:512], lhsT=lhsT, rhs=rhs0, start=start,
                          stop=stop)
    m2 = nc.tensor.matmul(ps[:, 512:1024], lhsT=lhsT, rhs=rhs1, start=start,
                          stop=stop)
    return m1.ins.name, m2.ins.name


def build_nc():
    nc = bacc.Bacc(None, target_bir_lowering=False)

    xT = nc.dram_tensor("xT", [D, BS], BF16, kind="ExternalInput")
    # weights host-pre-tiled to [P, DT*DC] so the DMA is a plain 2D copy
    w_in = {
        pr: nc.dram_tensor(f"w{pr}", [P, DT * DC], BF16, kind="ExternalInput")
        for pr in "qkv"
    }
    bqkv = nc.dram_tensor("bqkv", [DC, 3], F32, kind="ExternalInput")
    # mask host-pre-tiled to [P, B*KT] (key position on partitions)
    mkT = nc.dram_tensor("mkT", [P, B * KT], F32, kind="ExternalInput")
    out = nc.dram_tensor("out", [NU, W65, S], F32, kind="ExternalOutput")

    from contextlib import ExitStack

    with TileContext(nc) as tc, ExitStack() as es:
        const = es.enter_context(tc.tile_pool(name="const", bufs=1))
        wp = es.enter_context(tc.tile_pool(name="wsb", bufs=1))
        qkvp = es.enter_context(tc.tile_pool(name="qkv", bufs=1))
        xp = es.enter_context(tc.tile_pool(name="xsb", bufs=1))
        ptp = es.enter_context(tc.tile_pool(name="pt", bufs=4))
        obp = es.enter_context(tc.tile_pool(name="ob", bufs=2))

        ident = const.tile([P, P], BF16)
        make_identity(nc, ident)
        b_sb = const.tile([DC, 3], F32)
        mk = const.tile([P, B * KT], F32)
        w_sb = {
            pr: wp.tile([P, DT * DC], BF16, tag=f"w{pr}", name=f"w{pr}sb")
            for pr in "qkv"
        }
        # x^T staged in SBUF: one tile per (d-tile, batch, 1024-col half)
        xx = {}
        for b in range(B):
            for dt in range(DT):
                for h in range(2):
                    xx[(dt, b, h)] = xp.tile(
                        [P, SCH], BF16, tag=f"x{dt}_{b}_{h}",
                        name=f"x{dt}_{b}_{h}")

        def x_dma(dt, b, h):
            c0 = b * S + h * SCH
            nc.sync.dma_start(
                xx[(dt, b, h)][:], xT[dt * P:(dt + 1) * P, c0:c0 + SCH]
            )

        # DMA order matters: the sync queue serializes configs (~0.6us
        # each), so emit exactly what the first section needs first.
        nc.sync.dma_start(w_sb["q"][:], w_in["q"][:])
        for dt in range(DT):
            x_dma(dt, 0, 0)
            x_dma(dt, 0, 1)
        nc.sync.dma_start(w_sb["k"][:], w_in["k"][:])
        nc.sync.dma_start(w_sb["v"][:], w_in["v"][:])
        nc.sync.dma_start(b_sb[:], bqkv[:])
        for dt in range(DT):
            x_dma(dt, 1, 0)
            x_dma(dt, 1, 1)
        nc.sync.dma_start(mk[:], mkT[:])

        # Persistent per-core activations
        q_sb = qkvp.tile([P, BS], BF16)       # Q^T: [dq, (b s)]
        k_sb = qkvp.tile([P, BS], BF16)       # K^T
        v_t = qkvp.tile([P, BS], BF16)        # V^T staging (pre-transpose)
        vv = [
            qkvp.tile([P, KT * W130], BF16, tag=f"vv{b}", name=f"vv{b}")
            for b in range(B)
        ]

        def ones_memset(b):
            view = vv[b][:].rearrange("p (t g w) -> p t g w", g=2, w=W65)
            nc.vector.memset(view[:, :, :, W65 - 1:W65].squeeze(-1), 1.0)

        # ---------------- Phase A: projections + V layout ----------------
        with nc.named_scope("proj"):
            with tc.tile_pool(name="pp", bufs=3, space="PSUM") as pp, \
                 tc.tile_pool(name="pT", bufs=2, space="PSUM") as pT:

                def section(pr, b):
                    # both 1024-col chunks of (pr, batch b); dt-outer so the
                    # two chunks share each stationary load
                    ps = [
                        pp.tile([P, SCH], F32, tag="pp", name="pp")
                        for _ in range(2)
                    ]
                    for dt in range(DT):
                        for h in range(2):
                            _mm_pair(
                                nc, ps[h],
                                w_sb[pr][:, dt * DC:(dt + 1) * DC],
                                xx[(dt, b, h)][:, 0:512],
                                xx[(dt, b, h)][:, 512:SCH],
                                start=(dt == 0), stop=(dt == DT - 1),
                            )
                    for h in range(2):
                        sl = slice(b * S + h * SCH, b * S + (h + 1) * SCH)
                        if pr == "q":
                            nc.vector.tensor_scalar_add(
                                q_sb[:, sl], ps[h][:], b_sb[:, 0:1])
                        elif pr == "k":
                            nc.vector.tensor_scalar_add(
                                k_sb[:, sl], ps[h][:], b_sb[:, 1:2])
                        else:
                            nc.vector.tensor_copy(v_t[:, sl], ps[h][:])

                def vt_transposes(b):
                    for kt in range(KT):
                        tp = pT.tile([P, P], BF16, tag="tp", name="tp")
                        nc.tensor.transpose(
                            tp[:],
                            v_t[:, b * S + kt * P:b * S + (kt + 1) * P],
                            ident[:],
                        )
                        dst = vv[b][:, kt * W130:(kt + 1) * W130].rearrange(
                            "p (g w) -> p g w", w=W65)
                        nc.vector.tensor_copy(
                            dst[:, :, 0:HD],
                            tp[:].rearrange("p (g d) -> p g d", d=HD),
                        )

                ones_memset(0)
                ones_memset(1)
                for pr in "qkv":
                    section(pr, 0)
                vt_transposes(0)
                for pr in "qkv":
                    section(pr, 1)
                vt_transposes(1)

        # ---------------- Phase B: attention ----------------
        with nc.named_scope("attn"):
            with tc.tile_pool(name="sps", bufs=2, space="PSUM") as sp, \
                 tc.tile_pool(name="cps0", bufs=1, space="PSUM") as cp0, \
                 tc.tile_pool(name="cps1", bufs=1, space="PSUM") as cp1:
                cpools = [cp0, cp1]

                # The kt-outer loop leaves ~300ns/kt of PE slack behind
                # ScalarE's two exps; periodic idle gaps collapse the PE
                # p-state (2.4->1.2GHz) bistably. Early units' slack is
                # absorbed by the scheduler overlapping the projection tail;
                # later units keep the (otherwise deduped) redundant
                # stationary reloads of the lane-1 pairs as ~100ns of
                # dependency-free PE keep-warm work per site.
                keep_ld = []

                for u in range(NU):
                    b, hl = u // HPC, u % HPC
                    hp = slice(hl * HD, (hl + 1) * HD)
                    bs0 = b * S
                    cps = [
                        cpools[l].tile([W65, QH], F32, tag=f"cps{l}",
                                       name=f"cps{l}")
                        for l in range(NL)
                    ]
                    pts = [[None] * NL for _ in range(KT)]

                    def emit_ctx(j, u=u, b=b, hl=hl, cps=cps, pts=pts):
                        o0 = j * W130 + hl * W65
                        for l in range(NL):
                            names = _mm_pair(
                                nc, cps[l],
                                vv[b][:, o0:o0 + W65],
                                pts[j][l][:, 0:512],
                                pts[j][l][:, 512:1024],
                                start=(j == 0), stop=(j == KT - 1),
                            )
                            if l == 1 and (u >= 2 or (u == 1 and j >= 8)):
                                keep_ld.append(names[0])

                    for kt in range(KT):
                        sps = [sp.tile([P, QH], F32, tag="sps", name="sps")
                               for _ in range(NL)]
                        for l in range(NL):
                            q0 = bs0 + l * QH
                            names = _mm_pair(
                                nc, sps[l],
                                k_sb[hp, bs0 + kt * P:bs0 + (kt + 1) * P],
                                q_sb[hp, q0:q0 + 512],
                                q_sb[hp, q0 + 512:q0 + QH],
                                start=True, stop=True,
                            )
                            if l == 1 and u >= 2:
                                keep_ld.append(names[0])
                        if kt > 0:
                            emit_ctx(kt - 1)
                        for l in range(NL):
                            pt = ptp.tile([P, QH], BF16, tag="pt")
                            nc.scalar.activation(
                                pt[:], sps[l][:],
                                mybir.ActivationFunctionType.Exp,
                                bias=mk[:, b * KT + kt:b * KT + kt + 1],
                                scale=SCALE,
                            )
                            pts[kt][l] = pt
                    emit_ctx(KT - 1)
                    for l in range(NL):
                        ob = obp.tile([W65, QH], F32, tag="ob")
                        nc.vector.tensor_copy(ob[:], cps[l][:])
                        nc.sync.dma_start(
                            out[u, :, l * QH:(l + 1) * QH], ob[:])

    if DEDUPE_LDWEIGHTS:
        _dedupe_ldweights(nc, keep_ld)
    nc.compile()
    return nc


def _prep_in_maps(hidden_states, attention_mask, Wq, bq, Wk, bk, Wv, bv):
    bf = ml_dtypes.bfloat16
    hs = np.asarray(hidden_states, dtype=np.float32).reshape(BS, D)
    xT = np.ascontiguousarray(hs.T).astype(bf)
    # mask pre-tiled: mkT[p, b*KT + t] = mask[b, t*P + p]
    mkT = np.ascontiguousarray(
        np.asarray(attention_mask, dtype=np.float32).reshape(B, KT, P)
        .transpose(2, 0, 1).reshape(P, B * KT)
    )
    Ws = {"q": np.asarray(Wq, np.float32), "k": np.asarray(Wk, np.float32),
          "v": np.asarray(Wv, np.float32)}
    bs = {"q": np.asarray(bq, np.float32), "k": np.asarray(bk, np.float32),
          "v": np.asarray(bv, np.float32)}
    in_maps = []
    for c in range(N_CORES):
        sl = slice(c * DC, (c + 1) * DC)
        m = {"xT": xT, "mkT": mkT}
        for pr in "qkv":
            # pre-tiled: [P, DT*DC], column block dt = rows [dt*P,(dt+1)*P)
            wc = Ws[pr][:, sl].reshape(DT, P, DC).transpose(1, 0, 2)
            m[f"w{pr}"] = np.ascontiguousarray(wc.reshape(P, DT * DC)).astype(bf)
        m["bqkv"] = np.ascontiguousarray(
            np.stack([bs["q"][sl], bs["k"][sl], bs["v"][sl]], axis=1)
        )
        in_maps.append(m)
    return in_maps


def _gather(results, bv):
    bv = np.asarray(bv, np.float32)
    full = np.empty((B, S, D), dtype=np.float32)
    for c in range(N_CORES):
        o = results[c]["out"]  # [NU, 65, S] unnormalized ctx^T + denom row
        for b in range(B):
            for hl in range(HPC):
                u = b * HPC + hl
                col = c * DC + hl * HD
                ctx = o[u, :HD, :] / o[u, HD:HD + 1, :]
                full[b, :, col:col + HD] = ctx.T + bv[col:col + HD]
    return full


def kernel(hidden_states, attention_mask, Wq, bq, Wk, bk, Wv, bv, **run_kwargs):
    global _cached_nc
    if _cached_nc is None:
        _cached_nc = build_nc()
    in_maps = _prep_in_maps(
        hidden_states, attention_mask, Wq, bq, Wk, bk, Wv, bv
    )
    res = run_bass_kernel_spmd(
        _cached_nc, in_maps, core_ids=list(range(N_CORES)), **run_kwargs
    )
    full = _gather(res.results, bv)
    if run_kwargs:
        kernel.last_result = res
    return full



# revision 6
# speedup vs baseline: 1.1905x; 1.1905x over previous
"""BERT self-attention kernel for Trainium2, sharded over 8 NeuronCores.

Problem: nn_CustomBertSelfAttention (B=2, S=2048, D=1024, H=16 heads, HD=64).

Sharding: tensor-parallel over heads. Core c owns heads {2c, 2c+1}, i.e.
columns [128c, 128c+128) of Wq/Wk/Wv and of the output. Every core reads the
full hidden_states (transposed + cast to bf16 on the host).

Design notes (v2 — scheduled for HAM-warm density and an ACT-bound steady
state; the baseline lost ~126us to a sticky cold PE p-state and ~40us to a
phase-serialized lead-in):

  * All PSUM pools coexist (no pool open/close phase serialization):
    pj (proj + transposes + swaps, 1x[128,1024]f32 ring = 2 banks),
    sps (scores, 2x[128,1024]f32 = 4 banks), cps (ctx accum, 1x[65,1024]f32
    = 2 banks). Attention lanes are processed serially per unit so a single
    ctx accumulator suffices.
  * Projections are emitted per (proj, batch, 1024-token chunk) chasing the
    x DMA; batch-0 q,k first so the first exp fires at ~17us. Batch-1
    projection work is emitted interleaved into early attention pairs as PE
    filler (the steady state is ACT-bound, PE has slack).
  * Scores are computed in kt-PAIRS as two concurrent K=64 row-tiles of the
    PE array (rows 0-63 / 64-127, auto tile_position from base partitions).
    The odd tile sources Q^T/K^T from half-swapped copies (q_swap/k_swap)
    built with one PE matmul against a constant roll(eye(128),64) matrix.
    When the pipeline is ACT-bound the tiles serialize harmlessly; when the
    PE is the constraint (cold p-state / catch-up) they run concurrently,
    nearly halving scores time — cold-clock immunity.
  * exp on ScalarE with the attention mask folded in as the activation's
    per-partition bias (exact: exp(s*sc + m)); V is augmented with a ones
    column ([V|1], 65-wide stationary) so row 64 of the ctx accumulator is
    the softmax denominator. No on-device normalization: the raw [65, S]
    goes to DRAM and the host divides (and adds the V bias).
  * A post-build IR pass drops InstLdweights whose stationary is identical
    to the one already loaded.
Host: out[u] = (ctx[0:64] / ctx[64])^T + bv  gathered into [B, S, D].
"""
import sys

sys.path.insert(0, "/opt/trn_rl_repo")

import numpy as np
import ml_dtypes

from concourse import bacc
import concourse.mybir as mybir
from concourse.tile import TileContext
from concourse.masks import make_identity
from concourse.bass_utils import run_bass_kernel_spmd

B, S, D, H, HD = 2, 2048, 1024, 16, 64
N_CORES = 8
HPC = H // N_CORES          # heads per core = 2
DC = D // N_CORES           # output/weight columns per core = 128
BS = B * S                  # 4096
NU = B * HPC                # attention units per core = 4
P = 128
F32 = mybir.dt.float32
BF16 = mybir.dt.bfloat16
KT = S // P                 # 16 key tiles per unit
QH = 1024                   # query lane width
NL = S // QH                # 2 query lanes per unit
CH = 1024                   # projection token chunk
NC_CH = S // CH             # chunks per batch = 2
W65 = HD + 1                # V_aug width (V columns + ones column)
W130 = 2 * W65              # two heads interleaved per key tile in vv[b]
DT = D // P                 # 8 contraction tiles
SCALE = float(1.0 / np.sqrt(HD))

DEDUPE_LDWEIGHTS = True
FILLER_INTERLEAVE = True

_cached_nc = None


def _ap_key(arg):
    """Stable identity key for an LDWEIGHTS stationary access pattern."""
    try:
        bass_ap = getattr(arg, "bass_ap", None)
        if bass_ap is not None:
            return ("bap", bass_ap.tensor.name, bass_ap.offset,
                    tuple(map(tuple, bass_ap.ap)), str(arg.dtype))
        return ("raw", getattr(arg, "memref", ""), arg.offset,
                tuple(map(tuple, arg.ap)), str(arg.dtype))
    except Exception:
        return ("repr", repr(arg))


def _dedupe_ldweights(nc, keep_mm_names=()):
    """Drop PE weight reloads whose stationary is already in the array.

    The tile legalizer splits every InstMatmult into InstLdweights +
    InstMatmult. Runs of matmuls that share a stationary reload it
    redundantly; the PE array retains the stationary across matmuls, so
    duplicate loads are pure overhead (~100ns each). Dependencies carried
    by a dropped load are merged into the next PE instruction so no
    synchronization is lost. Operates on the post-scheduler order, so only
    loads that are genuinely redundant at execution time are removed.
    """
    keep = set(keep_mm_names)
    pe = mybir.EngineType.PE
    for f in nc.m.functions:
        for blk in f.blocks:
            insts = blk.instructions
            pe_seq = [i for i in insts
                      if getattr(i, "engine", None) == pe]
            next_mm_name = {}
            pending = []
            for i in pe_seq:
                if type(i).__name__ == "InstLdweights":
                    pending.append(i)
                else:
                    for ld in pending:
                        next_mm_name[id(ld)] = i.name
                    pending = []
            drop = set()
            cur_key = None
            pending_merge = []
            for i in pe_seq:
                tn = type(i).__name__
                if tn == "InstLdweights":
                    key = (
                        _ap_key(i.ins[0]),
                        getattr(i, "is_transpose", None),
                        getattr(i, "perf_mode", None),
                        getattr(i, "tile_position", None),
                    )
                    if key == cur_key and next_mm_name.get(id(i)) not in keep:
                        drop.add(id(i))
                        pending_merge.append(i)
                    else:
                        cur_key = key
                elif pending_merge:
                    for ld in pending_merge:
                        i.merge_dependencies_from(ld)
                    pending_merge = []
            if drop:
                blk.instructions = [i for i in insts if id(i) not in drop]


def build_nc():
    nc = bacc.Bacc(None, target_bir_lowering=False)

    xT = nc.dram_tensor("xT", [D, BS], BF16, kind="ExternalInput")
    # weights host-pre-tiled to [P, DT*DC] so the DMA is a plain 2D copy
    w_in = {
        pr: nc.dram_tensor(f"w{pr}", [P, DT * DC], BF16, kind="ExternalInput")
        for pr in "qkv"
    }
    bqkv = nc.dram_tensor("bqkv", [DC, 3], F32, kind="ExternalInput")
    # mask host-pre-tiled to [P, B*KT] (key position on partitions)
    mkT = nc.dram_tensor("mkT", [P, B * KT], F32, kind="ExternalInput")
    # half-swap constant: swap[i, j] = 1 iff j == (i+64) % 128
    swapT = nc.dram_tensor("swapT", [P, P], BF16, kind="ExternalInput")
    out = nc.dram_tensor("out", [NU, W65, S], F32, kind="ExternalOutput")

    from contextlib import ExitStack

    with TileContext(nc) as tc, ExitStack() as es:
        const = es.enter_context(tc.tile_pool(name="const", bufs=1))
        wp = es.enter_context(tc.tile_pool(name="wsb", bufs=1))
        xp = es.enter_context(tc.tile_pool(name="xsb", bufs=1))
        qk = es.enter_context(tc.tile_pool(name="qksb", bufs=1))
        ptp = es.enter_context(tc.tile_pool(name="pt", bufs=6))
        obp = es.enter_context(tc.tile_pool(name="ob", bufs=2))
        # PSUM: all three pools coexist (2 + 4 + 2 = 8 banks)
        pj = es.enter_context(tc.tile_pool(name="pj", bufs=1, space="PSUM"))
        sp = es.enter_context(tc.tile_pool(name="sps", bufs=2, space="PSUM"))
        cp = es.enter_context(tc.tile_pool(name="cps", bufs=1, space="PSUM"))

        ident = const.tile([P, P], BF16)
        make_identity(nc, ident)
        swap_sb = const.tile([P, P], BF16)
        b_sb = const.tile([DC, 3], F32)
        mk = const.tile([P, B * KT], F32)
        w_sb = {
            pr: wp.tile([P, DT * DC], BF16, tag=f"w{pr}", name=f"w{pr}sb")
            for pr in "qkv"
        }
        # x^T staged in SBUF: one tile per (d-tile, batch)
        xx = {
            (dt, b): xp.tile([P, S], BF16, tag=f"x{dt}_{b}", name=f"x{dt}_{b}")
            for b in range(B) for dt in range(DT)
        }

        # DMA order matters: the sync queue serializes transfers, so emit
        # exactly what the critical path needs first.
        nc.sync.dma_start(w_sb["q"][:], w_in["q"][:])
        nc.sync.dma_start(w_sb["k"][:], w_in["k"][:])
        nc.sync.dma_start(mk[:], mkT[:])
        nc.sync.dma_start(b_sb[:], bqkv[:])
        nc.sync.dma_start(swap_sb[:], swapT[:])
        for dt in range(DT):
            nc.sync.dma_start(xx[(dt, 0)][:], xT[dt * P:(dt + 1) * P, 0:S])
        nc.sync.dma_start(w_sb["v"][:], w_in["v"][:])
        for dt in range(DT):
            nc.sync.dma_start(xx[(dt, 1)][:], xT[dt * P:(dt + 1) * P, S:BS])

        # Persistent per-core activations.  q_sb/k_sb: [dq, tokens] with
        # head h on partitions [64h, 64h+64).  q_swap/k_swap: partition
        # halves swapped, so each head is also available in the *other*
        # half of the PE array for the kt-pair row-tiled scores.
        q_sb = [qk.tile([P, S], BF16, tag=f"qs{b}", name=f"qs{b}")
                for b in range(B)]
        k_sb = [qk.tile([P, S], BF16, tag=f"ks{b}", name=f"ks{b}")
                for b in range(B)]
        q_sw = [qk.tile([P, S], BF16, tag=f"qw{b}", name=f"qw{b}")
                for b in range(B)]
        k_sw = [qk.tile([P, S], BF16, tag=f"kw{b}", name=f"kw{b}")
                for b in range(B)]
        v_t = [qk.tile([P, S], BF16, tag=f"vt{b}", name=f"vt{b}")
               for b in range(B)]
        vv = [qk.tile([P, KT * W130], BF16, tag=f"vv{b}", name=f"vv{b}")
              for b in range(B)]

        for b in range(B):
            view = vv[b][:].rearrange("p (t g w) -> p t g w", g=2, w=W65)
            nc.vector.memset(view[:, :, :, W65 - 1:W65].squeeze(-1), 1.0)

        # ---------------- projection building blocks ----------------
        def proj_chunk(pr, b, c):
            """One [128, 1024] chunk of projection pr for batch b."""
            ps = pj.tile([P, CH], F32, tag="pp", name="pp")
            c0 = c * CH
            for dt in range(DT):
                w = w_sb[pr][:, dt * DC:(dt + 1) * DC]
                x = xx[(dt, b)]
                nc.tensor.matmul(ps[:, 0:512], w, x[:, c0:c0 + 512],
                                 start=(dt == 0), stop=(dt == DT - 1))
                nc.tensor.matmul(ps[:, 512:CH], w, x[:, c0 + 512:c0 + CH],
                                 start=(dt == 0), stop=(dt == DT - 1))
            if pr == "q":
                nc.vector.tensor_scalar_add(
                    q_sb[b][:, c0:c0 + CH], ps[:], b_sb[:, 0:1])
            elif pr == "k":
                nc.vector.tensor_scalar_add(
                    k_sb[b][:, c0:c0 + CH], ps[:], b_sb[:, 1:2])
            else:
                nc.vector.tensor_copy(v_t[b][:, c0:c0 + CH], ps[:])

        def swap_chunk(pr, b, c):
            """Half-swapped copy of a q/k chunk via PE: out = swap.T @ src."""
            src = q_sb[b] if pr == "q" else k_sb[b]
            dst = q_sw[b] if pr == "q" else k_sw[b]
            ps = pj.tile([P, CH], F32, tag="pp", name="pp")
            c0 = c * CH
            nc.tensor.matmul(ps[:, 0:512], swap_sb[:], src[:, c0:c0 + 512],
                             start=True, stop=True)
            nc.tensor.matmul(ps[:, 512:CH], swap_sb[:], src[:, c0 + 512:c0 + CH],
                             start=True, stop=True)
            nc.vector.tensor_copy(dst[:, c0:c0 + CH], ps[:])

        def transp_chunk(b, c):
            """PE-transpose 8 key tiles of V^T into the vv layout.

            The tp tile is bf16 [P, 2*CH] so its byte size matches the f32
            [P, CH] proj tiles sharing the "pp" pool slot; only the first
            CH columns are used.
            """
            tp = pj.tile([P, 2 * CH], BF16, tag="pp", name="pp")
            for j in range(8):
                kt = c * 8 + j
                nc.tensor.transpose(
                    tp[:, j * P:(j + 1) * P],
                    v_t[b][:, kt * P:(kt + 1) * P],
                    ident[:],
                )
            src = tp[:, 0:CH].rearrange("p (j g d) -> p j g d", g=2, d=HD)
            dst = vv[b][:, c * 8 * W130:(c + 1) * 8 * W130].rearrange(
                "p (j g w) -> p j g w", g=2, w=W65)
            nc.vector.tensor_copy(dst[:, :, :, 0:HD], src)

        def batch_work(b):
            """Projection + layout work for one batch, in dependency order."""
            items = []
            for c in range(NC_CH):
                items.append(lambda b=b, c=c: proj_chunk("q", b, c))
                items.append(lambda b=b, c=c: swap_chunk("q", b, c))
                items.append(lambda b=b, c=c: proj_chunk("k", b, c))
                items.append(lambda b=b, c=c: swap_chunk("k", b, c))
                items.append(lambda b=b, c=c: proj_chunk("v", b, c))
                items.append(lambda b=b, c=c: transp_chunk(b, c))
            return items

        # batch 0 up front (DMA-chasing); batch 1 becomes attention filler
        for item in batch_work(0):
            item()
        if FILLER_INTERLEAVE:
            filler = batch_work(1)
        else:
            for item in batch_work(1):
                item()
            filler = []
        fi = 0

        # ---------------- attention ----------------
        with nc.named_scope("attn"):
            n_emitted_pairs = 0
            for u in range(NU):
                b, h = u // HPC, u % HPC
                # head h's rows in the natural / swapped layouts
                nat = slice(h * HD, (h + 1) * HD)
                swp = slice((1 - h) * HD, (2 - h) * HD)
                for l in range(NL):
                    q0 = l * QH
                    cps = cp.tile([W65, QH], F32, tag="cps", name="cps")
                    pts = [None] * KT

                    def emit_ctx(kt, b=b, h=h, cps=cps, pts=pts):
                        o0 = kt * W130 + h * W65
                        va = vv[b][:, o0:o0 + W65]
                        nc.tensor.matmul(
                            cps[:, 0:512], va, pts[kt][:, 0:512],
                            start=(kt == 0), stop=(kt == KT - 1))
                        nc.tensor.matmul(
                            cps[:, 512:QH], va, pts[kt][:, 512:QH],
                            start=(kt == 0), stop=(kt == KT - 1))

                    for p in range(KT // 2):
                        ktE, ktO = 2 * p, 2 * p + 1
                        sE = sp.tile([P, QH], F32, tag="sps", name="sps")
                        sO = sp.tile([P, QH], F32, tag="sps", name="sps")
                        kE = k_sb[b][nat, ktE * P:(ktE + 1) * P]
                        kO = k_sw[b][swp, ktO * P:(ktO + 1) * P]
                        qE = q_sb[b][nat, q0:q0 + QH]
                        qO = q_sw[b][swp, q0:q0 + QH]
                        # interleave E/O so the two K=64 row-tiles can run
                        # concurrently when the PE is the constraint
                        nc.tensor.matmul(sE[:, 0:512], kE, qE[:, 0:512],
                                         start=True, stop=True)
                        nc.tensor.matmul(sO[:, 0:512], kO, qO[:, 0:512],
                                         start=True, stop=True)
                        nc.tensor.matmul(sE[:, 512:QH], kE, qE[:, 512:QH],
                                         start=True, stop=True)
                        nc.tensor.matmul(sO[:, 512:QH], kO, qO[:, 512:QH],
                                         start=True, stop=True)
                        if p > 0:
                            emit_ctx(ktE - 2)
                            emit_ctx(ktO - 2)
                        for kt, s in ((ktE, sE), (ktO, sO)):
                            pt = ptp.tile([P, QH], BF16, tag="pt")
                            nc.scalar.activation(
                                pt[:], s[:],
                                mybir.ActivationFunctionType.Exp,
                                bias=mk[:, b * KT + kt:b * KT + kt + 1],
                                scale=SCALE,
                            )
                            pts[kt] = pt
                        # batch-1 filler rides the ACT-bound PE slack
                        n_emitted_pairs += 1
                        if (fi < len(filler) and n_emitted_pairs >= 4
                                and n_emitted_pairs % 2 == 0):
                            filler[fi]()
                            fi += 1
                    emit_ctx(KT - 2)
                    emit_ctx(KT - 1)
                    ob = obp.tile([W65, QH], F32, tag="ob")
                    nc.vector.tensor_copy(ob[:], cps[:])
                    nc.sync.dma_start(out[u, :, q0:q0 + QH], ob[:])
            while fi < len(filler):
                filler[fi]()
                fi += 1

    if DEDUPE_LDWEIGHTS:
        _dedupe_ldweights(nc)
    nc.compile()
    return nc


def _prep_in_maps(hidden_states, attention_mask, Wq, bq, Wk, bk, Wv, bv):
    bf = ml_dtypes.bfloat16
    hs = np.asarray(hidden_states, dtype=np.float32).reshape(BS, D)
    xT = np.ascontiguousarray(hs.T).astype(bf)
    # mask pre-tiled: mkT[p, b*KT + t] = mask[b, t*P + p]
    mkT = np.ascontiguousarray(
        np.asarray(attention_mask, dtype=np.float32).reshape(B, KT, P)
        .transpose(2, 0, 1).reshape(P, B * KT)
    )
    swapT = np.ascontiguousarray(
        np.roll(np.eye(P, dtype=np.float32), P // 2, axis=1)).astype(bf)
    Ws = {"q": np.asarray(Wq, np.float32), "k": np.asarray(Wk, np.float32),
          "v": np.asarray(Wv, np.float32)}
    bs = {"q": np.asarray(bq, np.float32), "k": np.asarray(bk, np.float32),
          "v": np.asarray(bv, np.float32)}
    in_maps = []
    for c in range(N_CORES):
        sl = slice(c * DC, (c + 1) * DC)
        m = {"xT": xT, "mkT": mkT, "swapT": swapT}
        for pr in "qkv":
            # pre-tiled: [P, DT*DC], column block dt = rows [dt*P,(dt+1)*P)
            wc = Ws[pr][:, sl].reshape(DT, P, DC).transpose(1, 0, 2)
            m[f"w{pr}"] = np.ascontiguousarray(wc.reshape(P, DT * DC)).astype(bf)
        m["bqkv"] = np.ascontiguousarray(
            np.stack([bs["q"][sl], bs["k"][sl], bs["v"][sl]], axis=1)
        )
        in_maps.append(m)
    return in_maps


def _gather(results, bv):
    bv = np.asarray(bv, np.float32)
    full = np.empty((B, S, D), dtype=np.float32)
    for c in range(N_CORES):
        o = results[c]["out"]  # [NU, 65, S] unnormalized ctx^T + denom row
        for b in range(B):
            for hl in range(HPC):
                u = b * HPC + hl
                col = c * DC + hl * HD
                ctx = o[u, :HD, :] / o[u, HD:HD + 1, :]
                full[b, :, col:col + HD] = ctx.T + bv[col:col + HD]
    return full


def kernel(hidden_states, attention_mask, Wq, bq, Wk, bk, Wv, bv, **run_kwargs):
    global _cached_nc
    if _cached_nc is None:
        _cached_nc = build_nc()
    in_maps = _prep_in_maps(
        hidden_states, attention_mask, Wq, bq, Wk, bk, Wv, bv
    )
    res = run_bass_kernel_spmd(
        _cached_nc, in_maps, core_ids=list(range(N_CORES)), **run_kwargs
    )
    full = _gather(res.results, bv)
    if run_kwargs:
        kernel.last_result = res
    return full
